# revision 16
# baseline (speedup 1.0000x reference)
"""Multi-head attention kernel for Trainium2, sharded over 8 NeuronCores.

Sharding: core c handles batch c//4 and heads 4*(c%4) .. 4*(c%4)+4
(data parallel on batch, tensor parallel on heads).  Each core computes a
partial output projection (its heads' slice of Wout); the host sums the 4
partials per batch at gather time.

Self-contained: hardcodes B=2, S=2048, D=1024, H=16.
"""

import numpy as np
import ml_dtypes
from contextlib import ExitStack

import concourse.bass as bass
import concourse.tile as tile
from concourse import mybir
from concourse import bass_utils as _BU
from concourse.bass_utils import run_bass_kernel_spmd

# The default walrus invocation passes --enable-ldw-opt=false, which forces a
# serial LDWEIGHTS+MATMUL pair per matmul (~380 ns instead of ~215 ns at
# N=512).  Rewrite the flag so consecutive matmuls pipeline via the
# background weight buffer.
_orig_run_command = _BU.run_command


def _run_command_ldw(argv, **kw):
    argv = ["--enable-ldw-opt=true" if a == "--enable-ldw-opt=false" else a
            for a in argv]
    return _orig_run_command(argv, **kw)


_BU.run_command = _run_command_ldw

BF16 = ml_dtypes.bfloat16

B, S, D, H = 2, 2048, 1024, 16
A = 64                  # head dim
NH = 4                  # heads per core
M = NH * A              # 256: local projection width
SCALE = 1.0 / 32.0      # 1/sqrt(D)
P = 128
QC = 512                # q chunk (matmul free dim)
NQC = S // QC           # 4
KC = 128                # k chunk (contraction tile for PV)
NKC = S // KC           # 16
DC = D // P             # 8 d-chunks

F32 = mybir.dt.float32
DT16 = mybir.dt.bfloat16
EXP = mybir.ActivationFunctionType.Exp

_prog_cache = {}


def _bcast_part(ap, n):
    """Broadcast a [1, ...] AP across n partitions (step-0 partition dim)."""
    return bass.AP(tensor=ap.tensor, offset=ap.offset, ap=[[0, n]] + list(ap.ap[1:]))


def _build(causal: bool) -> bass.Bass:
    nc = bass.Bass()

    # all inputs pre-swizzled on host into SBUF layout (contiguous per
    # partition -> large DMA descriptors -> near-peak HBM bandwidth).
    # Activations are column-block-major so the q-chunk-0 slices can be
    # prioritized (the SDMA engines round-robin across queues at packet
    # granularity, so need-order within ONE queue is the only way to get
    # the prologue's data first).
    qT = nc.dram_tensor("qT", [P, NQC, DC, QC], DT16, kind="ExternalInput")
    cT = nc.dram_tensor("cT", [P, NQC, DC, QC], DT16, kind="ExternalInput")
    wqT = nc.dram_tensor("wqT", [P, DC, M], DT16, kind="ExternalInput")
    wkT = nc.dram_tensor("wkT", [P, DC, M], DT16, kind="ExternalInput")
    wvT = nc.dram_tensor("wvT", [P, DC, M], DT16, kind="ExternalInput")
    woT = nc.dram_tensor("woT", [P, 2, D], DT16, kind="ExternalInput")
    if causal:
        m01 = nc.dram_tensor("m01", [P, KC], DT16, kind="ExternalInput")
    else:
        emT = nc.dram_tensor("emT", [S, S], DT16, kind="ExternalInput")
    # bf16 output: host sums the 4 per-batch partials in f32; the ~0.4%
    # partial rounding is well inside the 2e-2 gate and halves store drain
    outT = nc.dram_tensor("outT", [D, S], DT16, kind="ExternalOutput")

    with tile.TileContext(nc) as tc, ExitStack() as ctx:
        const = ctx.enter_context(tc.tile_pool(name="const", bufs=1))

        # Persistent SBUF tensors
        qt_in = const.tile([P, DC, S], DT16, tag="qt_in")    # query^T  (d on partitions)
        ct_in = const.tile([P, DC, S], DT16, tag="ct_in")    # context^T
        wq_sb = const.tile([P, DC, M], DT16, tag="wq_sb")
        wk_sb = const.tile([P, DC, M], DT16, tag="wk_sb")
        wv_sb = const.tile([P, DC, M], DT16, tag="wv_sb")
        wo_sb = const.tile([P, 2, D], DT16, tag="wo_sb")
        qt = [const.tile([P, S], DT16, tag=f"qt{i}", name=f"qt{i}") for i in range(2)]   # Q^T m-chunks
        kt = [const.tile([P, S], DT16, tag=f"kt{i}", name=f"kt{i}") for i in range(2)]   # K^T m-chunks
        v_sb = const.tile([P, NKC, NH * (A + 1)], DT16, tag="v_sb")       # [V_h | 1] blocks
        u_sb = [const.tile([P, S], DT16, tag=f"u{i}", name=f"u{i}") for i in range(2)]  # normalized attn@V
        ones64 = const.tile([P, A], DT16, tag="ones64")  # lhsT for the Z partition-broadcast matmul
        if causal:
            m01_sb = const.tile([P, KC], DT16, tag="m01_sb")

        # Input DMAs: all on the sync queue, in need order.  A queue's ring
        # drains FIFO and one InstDMACopy spreads across all 16 SDMA engines,
        # so a single queue still hits ~350 GB/s while guaranteeing the
        # prologue's slices (wq, q/c chunk 0, wk, wv) finish first instead
        # of round-robin-sharing bandwidth with the whole 10 MB input set.
        nc.sync.dma_start(out=wq_sb[:], in_=wqT[:, :, :])
        nc.sync.dma_start(out=qt_in[:, :, 0:QC], in_=qT[:, 0, :, :])
        nc.sync.dma_start(out=wk_sb[:], in_=wkT[:, :, :])
        nc.sync.dma_start(out=ct_in[:, :, 0:QC], in_=cT[:, 0, :, :])
        nc.sync.dma_start(out=wv_sb[:], in_=wvT[:, :, :])
        if causal:
            nc.sync.dma_start(out=m01_sb[:], in_=m01[:, :])
        for qc in range(1, NQC):
            nc.sync.dma_start(out=ct_in[:, :, qc * QC:(qc + 1) * QC],
                              in_=cT[:, qc, :, :])
            nc.sync.dma_start(out=qt_in[:, :, qc * QC:(qc + 1) * QC],
                              in_=qT[:, qc, :, :])
            if qc == 1:
                nc.sync.dma_start(out=wo_sb[:], in_=woT[:, :, :])

        # ones columns for the Z (denominator) trick: only the 4 stripe
        # columns need setting (v_proj overwrites the rest)
        for h in range(NH):
            nc.vector.memset(v_sb[:, :, h * (A + 1) + A:h * (A + 1) + A + 1], 1.0)
        nc.vector.memset(ones64[:], 1.0)

        # ---- Fused projection + attention + output projection ---------------
        # Attention is ACT(exp)-throughput-bound; projections for query chunk
        # sc+1 are interleaved into attention(sc)'s emission so the PE fills
        # its exp-wait bubbles.  PSUM: proj/out-proj share 2 banks (tag ps_p),
        # scores 2x[128,1024]=4, pvA+pvB 2 -> 8 total.
        with tc.tile_pool(name="ps_proj", bufs=2, space="PSUM") as ps_proj, \
             tc.tile_pool(name="ps_s", bufs=2, space="PSUM") as ps_s_pool, \
             tc.tile_pool(name="ps_pv", bufs=1, space="PSUM") as ps_pv_pool, \
             tc.tile_pool(name="expool", bufs=10) as ex_pool, \
             tc.tile_pool(name="o_stage", bufs=8) as o_stage, \
             tc.tile_pool(name="norm", bufs=4) as norm_pool:

            def q_proj(mc, sc):
                ps = ps_proj.tile([P, QC], F32, tag="ps_p", name="ps_q")
                for dc_ in range(DC):
                    nc.tensor.matmul(
                        ps[:, 0:QC],
                        lhsT=wq_sb[:, dc_, mc * P:(mc + 1) * P],
                        rhs=qt_in[:, dc_, sc * QC:(sc + 1) * QC],
                        start=(dc_ == 0), stop=(dc_ == DC - 1),
                    )
                nc.vector.tensor_copy(out=qt[mc][:, sc * QC:(sc + 1) * QC], in_=ps[:, 0:QC])

            def k_proj(mc, sc):
                ps = ps_proj.tile([P, QC], F32, tag="ps_p", name="ps_k")
                for dc_ in range(DC):
                    nc.tensor.matmul(
                        ps[:, 0:QC],
                        lhsT=wk_sb[:, dc_, mc * P:(mc + 1) * P],
                        rhs=ct_in[:, dc_, sc * QC:(sc + 1) * QC],
                        start=(dc_ == 0), stop=(dc_ == DC - 1),
                    )
                nc.vector.tensor_copy(out=kt[mc][:, sc * QC:(sc + 1) * QC], in_=ps[:, 0:QC])

            def v_proj(cc):
                ps = ps_proj.tile([P, QC], F32, tag="ps_p", name="ps_v")
                for dc_ in range(DC):
                    nc.tensor.matmul(
                        ps[:, 0:M],
                        lhsT=ct_in[:, dc_, cc * P:(cc + 1) * P],
                        rhs=wv_sb[:, dc_, :],
                        start=(dc_ == 0), stop=(dc_ == DC - 1),
                    )
                for h in range(NH):
                    nc.vector.tensor_copy(
                        out=v_sb[:, cc, h * (A + 1):h * (A + 1) + A],
                        in_=ps[:, h * A:(h + 1) * A],
                    )

            def out_proj(jc, sc):
                if sc == NQC - 1 and jc % 2 == 1:
                    # attention is over; reuse a free scores bank
                    ps = ps_s_pool.tile([P, 2 * QC], F32, tag="ps_s", name="ps_o2")
                else:
                    ps = ps_proj.tile([P, QC], F32, tag="ps_p", name="ps_o")
                for ic in range(2):
                    nc.tensor.matmul(
                        ps[:, 0:QC],
                        lhsT=wo_sb[:, ic, jc * P:(jc + 1) * P],
                        rhs=u_sb[ic][:, sc * QC:(sc + 1) * QC],
                        start=(ic == 0), stop=(ic == 1),
                    )
                o_sb = o_stage.tile([P, QC], DT16, tag="o_sb")
                if jc % 2 == 0:
                    nc.scalar.copy(out=o_sb[:], in_=ps[:, 0:QC])
                else:
                    nc.vector.tensor_copy(out=o_sb[:], in_=ps[:, 0:QC])
                # stores stay off the sync queue (it carries the input stream)
                (nc.scalar if jc % 2 == 0 else nc.gpsimd).dma_start(
                    out=outT[:, :][jc * P:(jc + 1) * P, sc * QC:(sc + 1) * QC],
                    in_=o_sb[:])

            def attn_block(pr, sc, kc_, pvA, pvB, nkc):
                h0, h1 = 2 * pr, 2 * pr + 1
                r = kc_ - 4 * sc
                # Diagonal blocks: columns [0, 128r) are fully masked ->
                # skipped in scores/exp/PV.  The multiplicative mask only
                # touches the 128-wide boundary band (same j>=p triangle).
                w0 = KC * r if (causal and r > 0) else 0
                ps = ps_s_pool.tile([P, 2 * QC], F32, tag="ps_s", name="ps_s")
                nc.tensor.matmul(
                    ps[:, w0:QC],
                    lhsT=kt[pr][0:A, kc_ * KC:(kc_ + 1) * KC],
                    rhs=qt[pr][0:A, sc * QC + w0:(sc + 1) * QC],
                    start=True, stop=True,
                )
                nc.tensor.matmul(
                    ps[:, QC + w0:2 * QC],
                    lhsT=kt[pr][A:2 * A, kc_ * KC:(kc_ + 1) * KC],
                    rhs=qt[pr][A:2 * A, sc * QC + w0:(sc + 1) * QC],
                    start=True, stop=True,
                )
                ex = ex_pool.tile([P, 2 * QC], DT16, tag="ex", name="ex")
                # single activation per block; for diagonal blocks the span
                # [w0:2QC] also covers the never-read stale gap [QC:QC+w0]
                # (bounded scores -> exp stays finite), trading <=320ns of
                # extra ACT streaming for one instruction's ~330ns latency
                nc.scalar.activation(out=ex[:, w0:2 * QC], in_=ps[:, w0:2 * QC],
                                     func=EXP, scale=SCALE)
                if causal:
                    if r >= 0:  # mask the boundary band only
                        nc.vector.tensor_mul(
                            ex[:, w0:w0 + KC], ex[:, w0:w0 + KC], m01_sb[:])
                        nc.vector.tensor_mul(
                            ex[:, QC + w0:QC + w0 + KC],
                            ex[:, QC + w0:QC + w0 + KC], m01_sb[:])
                else:
                    em = ex_pool.tile([P, QC], DT16, tag="em", name="em")
                    nc.sync.dma_start(
                        out=em[:],
                        in_=emT[:, :][kc_ * KC:(kc_ + 1) * KC,
                                      sc * QC:(sc + 1) * QC],
                    )
                    nc.vector.tensor_mul(ex[:, 0:QC], ex[:, 0:QC], em[:])
                    nc.vector.tensor_mul(ex[:, QC:2 * QC], ex[:, QC:2 * QC], em[:])
                # PV with ones-column (psum row A holds Z); returned as a
                # closure so the caller can software-pipeline it one block
                # behind the next block's scores (keeps the PE FIFO from
                # stalling on the exp wait).
                def emit_pv():
                    nc.tensor.matmul(
                        pvA[0:A + 1, w0:QC],
                        lhsT=v_sb[:, kc_, h0 * (A + 1):(h0 + 1) * (A + 1)],
                        rhs=ex[:, w0:QC],
                        start=(kc_ == 0), stop=(kc_ == nkc - 1),
                    )
                    nc.tensor.matmul(
                        pvB[0:A + 1, w0:QC],
                        lhsT=v_sb[:, kc_, h1 * (A + 1):(h1 + 1) * (A + 1)],
                        rhs=ex[:, QC + w0:2 * QC],
                        start=(kc_ == 0), stop=(kc_ == nkc - 1),
                    )
                return emit_pv

            def normalize(pr, sc, pvA, pvB, tail=False):
                # tail=True: last chunk -> ACT engine is idle after the final
                # exp; route the copies there to dodge the DVE backlog.
                ceng = nc.scalar.copy if tail else nc.vector.tensor_copy
                # Evict U unnormalized (frees PV psum fast).  1/Z without any
                # DRAM bounce: Z rows (psum partition A) -> sbuf, partition-
                # broadcast via a K=1 matmul (PE is the idle engine here),
                # lane-parallel reciprocal, one in-place multiply.
                bt = norm_pool.tile([A, QC], DT16, tag="bt", name="bt")
                ceng(out=bt[:], in_=pvB[0:A, :])
                # partition shift h1 -> rows 64:128 via HWDGE (lower latency
                # than the gpsimd SW queue); the tail uses the idle sync queue
                (nc.sync if tail else nc.scalar).dma_start(
                    out=u_sb[pr][A:2 * A, sc * QC:(sc + 1) * QC], in_=bt[:])
                zr = norm_pool.tile([P, 2 * QC], DT16, tag="zr", name="zr")
                ceng(out=zr[A:A + 1, 0:QC], in_=pvA[A:A + 1, :])
                ceng(out=zr[A:A + 1, QC:2 * QC], in_=pvB[A:A + 1, :])
                rbp = ps_proj.tile([P, QC], F32, tag="ps_p", name="ps_rb")
                nc.tensor.matmul(rbp[0:A, :], lhsT=ones64[A:A + 1, :],
                                 rhs=zr[A:A + 1, 0:QC], start=True, stop=True)
                nc.tensor.matmul(rbp[A:2 * A, :], lhsT=ones64[A:A + 1, :],
                                 rhs=zr[A:A + 1, QC:2 * QC], start=True, stop=True)
                rb = norm_pool.tile([P, QC], F32, tag="rb", name="rb")
                nc.vector.reciprocal(out=rb[:], in_=rbp[:])
                ceng(out=u_sb[pr][0:A, sc * QC:(sc + 1) * QC], in_=pvA[0:A, :])
                nc.vector.tensor_mul(
                    u_sb[pr][:, sc * QC:(sc + 1) * QC],
                    u_sb[pr][:, sc * QC:(sc + 1) * QC], rb[:])

            # Prologue: only what attention(sc=0, pr=0) needs; pair-1
            # projections ride as the first fill units.
            q_proj(0, 0)
            k_proj(0, 0)
            for cc in range(4):
                v_proj(cc)
            if not causal:
                # non-causal attention reads all of K/V from chunk 0 on:
                # no interleave, project everything upfront
                for nsc in range(1, NQC):
                    for mc in range(2):
                        q_proj(mc, nsc)
                        k_proj(mc, nsc)
                for cc in range(4, NKC):
                    v_proj(cc)

            for sc in range(NQC):
                # PE filler units, balanced so the late (fill-starved but
                # ACT-exp-bound) chunks keep the PE busy: v blocks land in
                # the chunk that consumes them, q/k projections one chunk
                # ahead, and out-proj one chunk behind (which also gives the
                # normalize chain a whole chunk to finish off-critical-path).
                fill = []
                if causal and sc == 0:
                    fill.append(lambda: q_proj(1, 0))
                    fill.append(lambda: k_proj(1, 0))
                if causal and sc > 0:
                    for cc in range(4 * sc, min(4 * sc + 4, NKC)):
                        fill.append(lambda cc=cc: v_proj(cc))
                if causal and sc + 1 < NQC:
                    nsc = sc + 1
                    for mc in range(2):
                        fill.append(lambda mc=mc, nsc=nsc: q_proj(mc, nsc))
                        fill.append(lambda mc=mc, nsc=nsc: k_proj(mc, nsc))
                if causal and sc > 0:
                    for jc in range(D // P):
                        fill.append(lambda jc=jc, psc=sc - 1: out_proj(jc, psc))
                nkc = min(4 * sc + 4, NKC) if causal else NKC
                blocks = [(pr, kc_) for pr in range(2) for kc_ in range(nkc)]
                # spread fill units across this chunk's blocks; pop enough
                # per block that all are emitted in time (sc=0 needs 2/block)
                stride = max(1, len(blocks) // max(1, len(fill)))
                per_block = -(-len(fill) // len(blocks)) if fill else 0
                fi = 0
                pvt = {}
                pending = []   # deferred PV/normalize, one block behind scores
                for bi, (pr, kc_) in enumerate(blocks):
                    if kc_ == 0:
                        pvt[pr] = (
                            ps_pv_pool.tile([P, QC], F32, tag="pvA", name="pvA"),
                            ps_pv_pool.tile([P, QC], F32, tag="pvB", name="pvB"),
                        )
                    pv = attn_block(pr, sc, kc_, pvt[pr][0], pvt[pr][1], nkc)
                    if pending:
                        pending.pop(0)()
                    pending.append(pv)
                    if kc_ == nkc - 1:
                        pending.append(
                            lambda pr=pr, t=pvt[pr]: normalize(
                                pr, sc, t[0], t[1],
                                tail=(sc == NQC - 1 and pr == 1)))
                    if bi % stride == stride - 1:
                        for _ in range(per_block):
                            if fi < len(fill):
                                fill[fi]()
                                fi += 1
                while pending:
                    pending.pop(0)()
                while fi < len(fill):
                    fill[fi]()
                    fi += 1
                if not causal:
                    for jc in range(D // P):
                        out_proj(jc, sc)
            if causal:
                for jc in range(D // P):
                    out_proj(jc, NQC - 1)

    return nc


def _split_waits(nc: bass.Bass) -> int:
    """The walrus build here allows one sync wait per engine instruction;
    Tile emits several.  Hoist extras into standalone single-wait
    EventSemaphore instructions on the same engine queue (in-order, so
    semantics are preserved).  DMACopy waits lower into queue descriptors and
    are left alone."""
    n = 0
    for func in nc.m.functions:
        for block in func.blocks:
            out = []
            for ins in block.instructions:
                si = ins.sync_info
                if si is not None and len(si.on_wait) > 1:
                    waits = list(si.on_wait)
                    for w in waits[:-1]:
                        es = mybir.InstEventSemaphore(
                            name=f"waitsplit_{n}", ins=[], outs=[])
                        n += 1
                        es.engine = ins.engine
                        es.sync_info = type(si)(on_wait=[w], on_update=[])
                        out.append(es)
                    si.on_wait = [waits[-1]]
                    ins.sync_info = si
                out.append(ins)
            block.instructions = out
    return n


def _fuse_ldweights(nc: bass.Bass) -> int:
    """walrus's --enable-ldw-opt (background weight loading into the PE's
    second weight buffer, overlapped with the running matmul) rejects ANY
    explicit InstLdweights (CoreV3GenImpl::visitInstLdweights asserts
    !enableLDWOpt unconditionally).  tile_legalize always splits bf16
    matmuls into LDW+MM pairs, so undo that: drop the InstLdweights and
    mark each InstMatmult self-loading (ldweights=True) — walrus then
    emits its own background-load form.  The few waits parked on LDWs by
    move_matmul_waits_to_ldweights become standalone EventSemaphore
    instructions (same PE queue, in-order, so semantics are preserved)."""
    n = 0
    for func in nc.m.functions:
        for block in func.blocks:
            out = []
            for ins in block.instructions:
                if isinstance(ins, mybir.InstLdweights):
                    si = ins.sync_info
                    if si is not None and (si.on_wait or si.on_update):
                        assert not si.on_update, "LDW with updates unexpected"
                        for w in si.on_wait:
                            es = mybir.InstEventSemaphore(
                                name=f"ldwsync_{n}", ins=[], outs=[])
                            n += 1
                            es.engine = ins.engine
                            es.sync_info = type(si)(on_wait=[w], on_update=[])
                            out.append(es)
                    continue  # drop the LDW itself
                if isinstance(ins, mybir.InstMatmult):
                    ins.ldweights = True
                out.append(ins)
            block.instructions = out
    return n


def _get_prog(causal: bool) -> bass.Bass:
    if causal not in _prog_cache:
        nc = _build(causal)
        _split_waits(nc)
        _fuse_ldweights(nc)
        _prog_cache[causal] = nc
    return _prog_cache[causal]


def _is_causal(mask: np.ndarray) -> bool:
    if mask.shape != (S, S):
        return False
    tri = np.tril(np.ones((S, S), dtype=bool))
    low = mask[tri]
    up = mask[~tri]
    return bool((low == 0.0).all() and (up <= -1e8).all())


def _m01_patterns() -> np.ndarray:
    # Boundary-band mask: band column j vs partition p -> keep iff j >= p.
    j = np.arange(KC)[None, :]
    p = np.arange(P)[:, None]
    return (j >= p).astype(BF16)


def _prep_in_maps(query, context, Wq, Wkv, Wout, mask, causal):
    query = np.asarray(query, dtype=np.float32)
    context = np.asarray(context, dtype=np.float32)
    Wq = np.asarray(Wq, dtype=np.float32)
    Wkv = np.asarray(Wkv, dtype=np.float32)
    Wout = np.asarray(Wout, dtype=np.float32)

    def sw_act(x):   # [D, S] -> [P, NQC, DC, QC] (SBUF-layout, q-chunk-major)
        return np.ascontiguousarray(
            x.reshape(DC, P, NQC, QC).transpose(1, 2, 0, 3)).astype(BF16)

    def sw_w(w):     # [D, M] -> [P, DC, M]
        return np.ascontiguousarray(
            w.reshape(DC, P, M).transpose(1, 0, 2)).astype(BF16)

    def sw_wo(w):    # [M, D] -> [P, 2, D]
        return np.ascontiguousarray(
            w.reshape(2, P, D).transpose(1, 0, 2)).astype(BF16)

    qT = [sw_act(query[b].T) for b in range(B)]
    cT = [sw_act(context[b].T) for b in range(B)]
    if causal:
        extra = ("m01", _m01_patterns())
    else:
        extra = ("emT", np.exp((SCALE * np.asarray(mask, np.float32).T)).astype(BF16))

    in_maps = []
    for c in range(8):
        b, g = divmod(c, 4)
        m0 = g * M
        in_maps.append({
            "qT": qT[b],
            "cT": cT[b],
            "wqT": sw_w(Wq[m0:m0 + M, :].T),
            "wkT": sw_w(Wkv[m0:m0 + M, :].T),
            "wvT": sw_w(Wkv[D + m0:D + m0 + M, :].T),
            "woT": sw_wo(Wout[:, m0:m0 + M].T),
            extra[0]: extra[1],
        })
    return in_maps


def _run(query, context, Wq, Wkv, Wout, mask, trace=False):
    causal = _is_causal(np.asarray(mask, np.float32))
    in_maps = _prep_in_maps(query, context, Wq, Wkv, Wout, mask, causal)
    nc = _get_prog(causal)
    res = run_bass_kernel_spmd(nc, in_maps, list(range(8)), trace=trace)
    out = np.zeros((B, S, D), dtype=np.float32)
    for c in range(8):
        out[c // 4] += res.results[c]["outT"].astype(np.float32).T
    return out, res


def kernel(query, context, Wq, Wkv, Wout, mask):
    out, _ = _run(query, context, Wq, Wkv, Wout, mask, trace=False)
    return out

